# revision 3
# baseline (speedup 1.0000x reference)
"""DisplaceChannel kernel for Trainium2 (8 NeuronCores, Bass/Tile).

out = depthwise3x3(displace(inp, round(offset)), gaussian(offset - round(offset)))

Strategy:
- Data-parallel over batch: 32 batches -> 4 per core.
- Integer displacement folded into the input DMA: each position's valid
  sub-rectangle is copied into a zero-initialized padded SBUF tile at the
  displaced location (offsets are host-computed compile-time constants).
- The 3x3 Gaussian kernel is exactly separable (exp(-(dx^2+dy^2)) =
  exp(-dx^2)exp(-dy^2), normalization factorizes), so the depthwise conv is a
  3-tap y-conv then a 3-tap x-conv. Each stage is one per-partition-scalar
  multiply on ScalarE plus two fused multiply-accumulate
  (scalar_tensor_tensor) ops on VectorE.
- Positions are packed 4 per tile (4 pos x 4 batch x 8 chan = 128 partitions),
  grouped by equal y-offset so each tile only computes its nonzero row band.
- Output HBM buffer is pre-zeroed by the runtime; only each position's
  nonzero band is DMA'd out.
"""
import os
import sys

import numpy as np

for _p in ("/opt/trn_rl_repo", "/root/.axon_site/_ro/trn_rl_repo"):
    if os.path.isdir(_p) and _p not in sys.path:
        sys.path.insert(0, _p)
        break

from contextlib import ExitStack

import concourse.bass as bass  # noqa: F401  (import keeps package init consistent)
import concourse.tile as tile
from concourse import bacc, mybir
from concourse.bass_utils import run_bass_kernel_spmd

H = 64
W = 64
B = 32
CHAN_PER_POS = 8
NUM_POS = 48
C = NUM_POS * CHAN_PER_POS
SIGMA = 0.5
NCORES = 8
BL = B // NCORES  # local batches per core
POS_PER_GROUP = 4
F32 = mybir.dt.float32

_cache = {}


def _geometry(offset):
    """Host-side: integer offsets, separable taps, per-position rects/bands."""
    off_round = np.round(offset)  # round-half-even, matches jnp.round
    oxy = off_round.astype(np.int64)
    frac = (offset - off_round).astype(np.float32)

    coords = (np.arange(3, dtype=np.float32) - np.float32(1.0))
    dx = coords[None, :] + frac[:, 0:1]
    dy = coords[None, :] + frac[:, 1:2]
    inv = np.float32(1.0 / (2.0 * SIGMA * SIGMA))
    gx = np.exp(-(dx * dx) * inv).astype(np.float32)
    gy = np.exp(-(dy * dy) * inv).astype(np.float32)
    wx = gx / gx.sum(axis=1, keepdims=True)
    wy = gy / gy.sum(axis=1, keepdims=True)

    pos = []
    for p in range(NUM_POS):
        ox, oy = int(oxy[p, 0]), int(oxy[p, 1])
        vy0, vy1 = max(0, oy), min(H, H + oy)
        vx0, vx1 = max(0, ox), min(W, W + ox)
        if vy1 <= vy0 or vx1 <= vx0:
            pos.append(None)
            continue
        pos.append(dict(
            p=p, ox=ox, oy=oy,
            vy0=vy0, vy1=vy1, vx0=vx0, vx1=vx1,
            sy0=vy0 - oy, sx0=vx0 - ox,
            by0=max(0, vy0 - 1), by1=min(H, vy1 + 1),
            bx0=max(0, vx0 - 1), bx1=min(W, vx1 + 1),
        ))

    live = [p for p in range(NUM_POS) if pos[p] is not None]
    live.sort(key=lambda p: (pos[p]["oy"], pos[p]["ox"]))
    groups = []
    for i in range(0, len(live), POS_PER_GROUP):
        members = [pos[p] for p in live[i:i + POS_PER_GROUP]]
        gby0 = min(m["by0"] for m in members)
        gby1 = max(m["by1"] for m in members)
        groups.append(dict(members=members, gby0=gby0, gby1=gby1))

    ng = len(groups)
    taps = np.zeros((128, max(ng, 1) * 6), dtype=np.float32)
    for g, grp in enumerate(groups):
        for i, m in enumerate(grp["members"]):
            rows = slice(i * 32, (i + 1) * 32)
            for k in range(3):
                taps[rows, g * 6 + k] = wy[m["p"], k]
                taps[rows, g * 6 + 3 + k] = wx[m["p"], k]
    return groups, taps


def _build(groups, n_tap_cols):
    nc = bacc.Bacc("TRN2", target_bir_lowering=False, debug=False,
                   num_devices=NCORES)
    inp_d = nc.dram_tensor("inp", [BL, C, H, W], F32, kind="ExternalInput")
    taps_d = nc.dram_tensor("taps", [128, n_tap_cols], F32, kind="ExternalInput")
    out_d = nc.dram_tensor("out", [BL, C, H, W], F32, kind="ExternalOutput")

    WP = W + 2  # padded row width

    mult = mybir.AluOpType.mult
    add = mybir.AluOpType.add

    with tile.TileContext(nc) as tc:
        with ExitStack() as ctx:
            dpool = ctx.enter_context(tc.tile_pool(name="dpool", bufs=3))
            tpool = ctx.enter_context(tc.tile_pool(name="tpool", bufs=2))
            opool = ctx.enter_context(tc.tile_pool(name="opool", bufs=2))
            cpool = ctx.enter_context(tc.tile_pool(name="cpool", bufs=1))

            taps_t = cpool.tile([128, n_tap_cols], F32, tag="taps")
            nc.sync.dma_start(taps_t[:], taps_d.ap()[:, :])

            def tap(g, k):
                return taps_t[:, g * 6 + k:g * 6 + k + 1]

            for g, grp in enumerate(groups):
                gby0, gby1 = grp["gby0"], grp["gby1"]
                bg = gby1 - gby0
                drows = bg + 2

                d_t = dpool.tile([128, drows * WP], F32, tag="D")
                nc.gpsimd.memset(d_t[:], 0.0)
                d3 = d_t[:].rearrange("q (r c) -> q r c", c=WP)
                for i, m in enumerate(grp["members"]):
                    hv = m["vy1"] - m["vy0"]
                    wv = m["vx1"] - m["vx0"]
                    r0 = 1 + m["vy0"] - gby0
                    c0 = 1 + m["vx0"]
                    for b in range(BL):
                        q0 = i * 32 + b * 8
                        dst = d3[q0:q0 + 8, r0:r0 + hv, c0:c0 + wv]
                        src = inp_d.ap()[b, 8 * m["p"]:8 * m["p"] + 8,
                                         m["sy0"]:m["sy0"] + hv,
                                         m["sx0"]:m["sx0"] + wv]
                        nc.sync.dma_start(dst, src)

                # y-conv: T[r, c] = sum_ky wy[ky] * D[r + ky, c]
                t_t = tpool.tile([128, bg * WP], F32, tag="T")
                nc.scalar.mul(t_t[:], d_t[:, 0:bg * WP], tap(g, 0))
                nc.vector.scalar_tensor_tensor(
                    t_t[:], d_t[:, WP:WP + bg * WP], tap(g, 1), t_t[:],
                    mult, add)
                nc.vector.scalar_tensor_tensor(
                    t_t[:], d_t[:, 2 * WP:2 * WP + bg * WP], tap(g, 2), t_t[:],
                    mult, add)

                # x-conv: O[r, x] = sum_kx wx[kx] * T[r, x + kx]
                o_t = opool.tile([128, bg * W], F32, tag="O")
                t3 = t_t[:].rearrange("q (r c) -> q r c", c=WP)
                o3 = o_t[:].rearrange("q (r c) -> q r c", c=W)
                nc.scalar.mul(o3[:, :, :], t3[:, :, 0:W], tap(g, 3))
                nc.vector.scalar_tensor_tensor(
                    o3[:, :, :], t3[:, :, 1:1 + W], tap(g, 4), o3[:, :, :],
                    mult, add)
                nc.vector.scalar_tensor_tensor(
                    o3[:, :, :], t3[:, :, 2:2 + W], tap(g, 5), o3[:, :, :],
                    mult, add)

                for i, m in enumerate(grp["members"]):
                    for b in range(BL):
                        q0 = i * 32 + b * 8
                        src = o3[q0:q0 + 8,
                                 m["by0"] - gby0:m["by1"] - gby0,
                                 m["bx0"]:m["bx1"]]
                        dst = out_d.ap()[b, 8 * m["p"]:8 * m["p"] + 8,
                                         m["by0"]:m["by1"], m["bx0"]:m["bx1"]]
                        nc.sync.dma_start(dst, src)

    nc.compile()
    return nc


def kernel(inp, offset):
    inp = np.ascontiguousarray(inp, dtype=np.float32)
    offset = np.ascontiguousarray(offset, dtype=np.float32)
    assert inp.shape == (B, C, H, W), inp.shape

    key = offset.tobytes()
    if key not in _cache:
        groups, taps = _geometry(offset)
        nc = _build(groups, taps.shape[1])
        _cache[key] = (nc, taps)
    nc, taps = _cache[key]

    in_maps = [{"inp": inp[c * BL:(c + 1) * BL], "taps": taps}
               for c in range(NCORES)]
    trace = os.environ.get("KERNEL_TRACE", "") == "1"
    res = run_bass_kernel_spmd(nc, in_maps, core_ids=list(range(NCORES)),
                               trace=trace)
    if trace:
        print(f"HW exec time: {res.exec_time_ns} ns "
              f"(mean {res.mean_exec_time_ns})")
        kernel.last_exec_time_ns = res.exec_time_ns
    out = np.concatenate([res.results[c]["out"] for c in range(NCORES)],
                         axis=0)
    return out


# revision 6
# speedup vs baseline: 1.2775x; 1.2775x over previous
"""DisplaceChannel kernel for Trainium2 (8 NeuronCores, Bass/Tile).

out = depthwise3x3(displace(inp, round(offset)), gaussian(offset - round(offset)))

Strategy:
- Data-parallel over batch: 32 batches -> 4 per core.
- Integer displacement folded into the input DMA: each position's valid
  sub-rectangle is copied into a zero-initialized padded SBUF tile at the
  displaced location (offsets are host-computed compile-time constants).
- The 3x3 Gaussian kernel is exactly separable (exp(-(dx^2+dy^2)) =
  exp(-dx^2)exp(-dy^2), normalization factorizes), so the depthwise conv is a
  3-tap y-conv then a 3-tap x-conv. Each stage is one per-partition-scalar
  multiply on ScalarE plus two fused multiply-accumulate
  (scalar_tensor_tensor) ops on VectorE.
- Positions are packed 4 per tile (4 pos x 4 batch x 8 chan = 128 partitions),
  grouped by equal y-offset so each tile only computes its nonzero row band.
- Output HBM buffer is pre-zeroed by the runtime; only each position's
  nonzero band is DMA'd out.
"""
import os
import sys

import numpy as np

for _p in ("/opt/trn_rl_repo", "/root/.axon_site/_ro/trn_rl_repo"):
    if os.path.isdir(_p) and _p not in sys.path:
        sys.path.insert(0, _p)
        break

from contextlib import ExitStack

import concourse.bass as bass  # noqa: F401  (import keeps package init consistent)
import concourse.tile as tile
from concourse import bacc, mybir
from concourse.bass_utils import run_bass_kernel_spmd

H = 64
W = 64
B = 32
CHAN_PER_POS = 8
NUM_POS = 48
C = NUM_POS * CHAN_PER_POS
SIGMA = 0.5
NCORES = 8
BL = B // NCORES  # local batches per core
POS_PER_GROUP = 4
F32 = mybir.dt.float32

_cache = {}


def _geometry(offset):
    """Host-side: integer offsets, separable taps, per-position rects/bands."""
    off_round = np.round(offset)  # round-half-even, matches jnp.round
    oxy = off_round.astype(np.int64)
    frac = (offset - off_round).astype(np.float32)

    coords = (np.arange(3, dtype=np.float32) - np.float32(1.0))
    dx = coords[None, :] + frac[:, 0:1]
    dy = coords[None, :] + frac[:, 1:2]
    inv = np.float32(1.0 / (2.0 * SIGMA * SIGMA))
    gx = np.exp(-(dx * dx) * inv).astype(np.float32)
    gy = np.exp(-(dy * dy) * inv).astype(np.float32)
    wx = gx / gx.sum(axis=1, keepdims=True)
    wy = gy / gy.sum(axis=1, keepdims=True)

    pos = []
    for p in range(NUM_POS):
        ox, oy = int(oxy[p, 0]), int(oxy[p, 1])
        vy0, vy1 = max(0, oy), min(H, H + oy)
        vx0, vx1 = max(0, ox), min(W, W + ox)
        if vy1 <= vy0 or vx1 <= vx0:
            pos.append(None)
            continue
        pos.append(dict(
            p=p, ox=ox, oy=oy,
            vy0=vy0, vy1=vy1, vx0=vx0, vx1=vx1,
            sy0=vy0 - oy, sx0=vx0 - ox,
            by0=max(0, vy0 - 1), by1=min(H, vy1 + 1),
            bx0=max(0, vx0 - 1), bx1=min(W, vx1 + 1),
        ))

    live = [p for p in range(NUM_POS) if pos[p] is not None]
    live.sort(key=lambda p: (pos[p]["oy"], pos[p]["ox"]))
    groups = []
    for i in range(0, len(live), POS_PER_GROUP):
        members = [pos[p] for p in live[i:i + POS_PER_GROUP]]
        gby0 = min(m["by0"] for m in members)
        gby1 = max(m["by1"] for m in members)
        groups.append(dict(members=members, gby0=gby0, gby1=gby1))

    ng = len(groups)
    taps = np.zeros((128, max(ng, 1) * 6), dtype=np.float32)
    for g, grp in enumerate(groups):
        for i, m in enumerate(grp["members"]):
            rows = slice(i * 32, (i + 1) * 32)
            for k in range(3):
                taps[rows, g * 6 + k] = wy[m["p"], k]
                taps[rows, g * 6 + 3 + k] = wx[m["p"], k]
    return groups, taps


def _build(groups, n_tap_cols):
    nc = bacc.Bacc("TRN2", target_bir_lowering=False, debug=False,
                   num_devices=NCORES)
    inp_d = nc.dram_tensor("inp", [BL, C, H, W], F32, kind="ExternalInput")
    taps_d = nc.dram_tensor("taps", [128, n_tap_cols], F32, kind="ExternalInput")
    out_d = nc.dram_tensor("out", [BL, C, H, W], F32, kind="ExternalOutput")

    WP = W + 2  # padded row width

    mult = mybir.AluOpType.mult
    add = mybir.AluOpType.add

    with tile.TileContext(nc) as tc:
        with ExitStack() as ctx:
            dpool = ctx.enter_context(tc.tile_pool(name="dpool", bufs=3))
            tpool = ctx.enter_context(tc.tile_pool(name="tpool", bufs=2))
            opool = ctx.enter_context(tc.tile_pool(name="opool", bufs=2))
            cpool = ctx.enter_context(tc.tile_pool(name="cpool", bufs=1))

            taps_t = cpool.tile([128, n_tap_cols], F32, tag="taps")
            nc.sync.dma_start(taps_t[:], taps_d.ap()[:, :])

            def tap(g, k):
                return taps_t[:, g * 6 + k:g * 6 + k + 1]

            for g, grp in enumerate(groups):
                gby0, gby1 = grp["gby0"], grp["gby1"]
                bg = gby1 - gby0
                drows = bg + 2

                d_t = dpool.tile([128, drows * WP], F32, tag="D")
                nc.gpsimd.memset(d_t[:], 0.0)
                d3 = d_t[:].rearrange("q (r c) -> q r c", c=WP)
                for i, m in enumerate(grp["members"]):
                    hv = m["vy1"] - m["vy0"]
                    wv = m["vx1"] - m["vx0"]
                    r0 = 1 + m["vy0"] - gby0
                    c0 = 1 + m["vx0"]
                    for b in range(BL):
                        q0 = i * 32 + b * 8
                        dst = d3[q0:q0 + 8, r0:r0 + hv, c0:c0 + wv]
                        src = inp_d.ap()[b, 8 * m["p"]:8 * m["p"] + 8,
                                         m["sy0"]:m["sy0"] + hv,
                                         m["sx0"]:m["sx0"] + wv]
                        eng = nc.sync if (i + b) % 2 == 0 else nc.scalar
                        eng.dma_start(dst, src)

                # y-conv: T[r, c] = sum_ky wy[ky] * D[r + ky, c]
                t_t = tpool.tile([128, bg * WP], F32, tag="T")
                nc.scalar.mul(t_t[:], d_t[:, 0:bg * WP], tap(g, 0))
                nc.vector.scalar_tensor_tensor(
                    t_t[:], d_t[:, WP:WP + bg * WP], tap(g, 1), t_t[:],
                    mult, add)
                nc.vector.scalar_tensor_tensor(
                    t_t[:], d_t[:, 2 * WP:2 * WP + bg * WP], tap(g, 2), t_t[:],
                    mult, add)

                # x-conv: O[r, x] = sum_kx wx[kx] * T[r, x + kx]
                o_t = opool.tile([128, bg * W], F32, tag="O")
                t3 = t_t[:].rearrange("q (r c) -> q r c", c=WP)
                o3 = o_t[:].rearrange("q (r c) -> q r c", c=W)
                nc.scalar.mul(o3[:, :, :], t3[:, :, 0:W], tap(g, 3))
                nc.vector.scalar_tensor_tensor(
                    o3[:, :, :], t3[:, :, 1:1 + W], tap(g, 4), o3[:, :, :],
                    mult, add)
                nc.vector.scalar_tensor_tensor(
                    o3[:, :, :], t3[:, :, 2:2 + W], tap(g, 5), o3[:, :, :],
                    mult, add)

                # full-width rows: O is exactly zero outside each position's
                # x-band, so writing all 64 cols keeps rows contiguous in HBM
                # (one ~rows*256B chunk per channel instead of per-row packets)
                for i, m in enumerate(grp["members"]):
                    r0, r1 = m["by0"] - gby0, m["by1"] - gby0
                    for b in range(BL):
                        q0 = i * 32 + b * 8
                        src = o_t[q0:q0 + 8, r0 * W:r1 * W]
                        dst = out_d.ap()[b, 8 * m["p"]:8 * m["p"] + 8,
                                         m["by0"]:m["by1"], :]
                        dst = dst.rearrange("ch r c -> ch (r c)")
                        eng = nc.sync if (i + b) % 2 == 0 else nc.scalar
                        eng.dma_start(dst, src)

    nc.compile()
    return nc


def kernel(inp, offset):
    inp = np.ascontiguousarray(inp, dtype=np.float32)
    offset = np.ascontiguousarray(offset, dtype=np.float32)
    assert inp.shape == (B, C, H, W), inp.shape

    key = offset.tobytes()
    if key not in _cache:
        groups, taps = _geometry(offset)
        nc = _build(groups, taps.shape[1])
        _cache[key] = (nc, taps)
    nc, taps = _cache[key]

    in_maps = [{"inp": inp[c * BL:(c + 1) * BL], "taps": taps}
               for c in range(NCORES)]
    trace = os.environ.get("KERNEL_TRACE", "") == "1"
    res = run_bass_kernel_spmd(nc, in_maps, core_ids=list(range(NCORES)),
                               trace=trace)
    if trace:
        print(f"HW exec time: {res.exec_time_ns} ns "
              f"(mean {res.mean_exec_time_ns})")
        kernel.last_exec_time_ns = res.exec_time_ns
    out = np.concatenate([res.results[c]["out"] for c in range(NCORES)],
                         axis=0)
    return out


# revision 7
# speedup vs baseline: 1.9186x; 1.5019x over previous
"""DisplaceChannel kernel for Trainium2 (8 NeuronCores, Bass/Tile).

out = depthwise3x3(displace(inp, round(offset)), gaussian(offset - round(offset)))

Strategy (v4):
- Data-parallel over batch: 32 batches -> 4 per core.
- Positions are packed 4 per tile (4 pos x 4 batch x 8 chan = 128
  partitions), grouped by EQUAL integer x-offset (sorted by y-offset within
  the group, so the row-band union stays tight).
- The y-displacement is folded into the input DMA row placement; rows are
  copied FULL-WIDTH so each (channel) transfer is one contiguous
  Hv*256B chunk (no per-row packet fragmentation).
- The x-displacement is folded into the x-conv access-pattern offsets
  (uniform within a group because groups share ox). The data sits in
  unshifted "u" coordinates until the x-conv writes shifted output
  coordinates.
- The 3x3 Gaussian kernel is exactly separable, so the depthwise conv is a
  3-tap y-conv then a 3-tap x-conv: per stage one per-partition-scalar
  multiply on ScalarE + two fused multiply-accumulates
  (scalar_tensor_tensor) on VectorE, restricted to each group's
  row-band x col-window.
- Output HBM is pre-zeroed by the runtime; each position's nonzero row band
  is written full-width (contiguous), with zeros in the off-band columns.
"""
import os
import sys

import numpy as np

for _p in ("/opt/trn_rl_repo", "/root/.axon_site/_ro/trn_rl_repo"):
    if os.path.isdir(_p) and _p not in sys.path:
        sys.path.insert(0, _p)
        break

from contextlib import ExitStack

import concourse.bass as bass  # noqa: F401
import concourse.tile as tile
from concourse import bacc, mybir
from concourse.bass_utils import run_bass_kernel_spmd

H = 64
W = 64
B = 32
CHAN_PER_POS = 8
NUM_POS = 48
C = NUM_POS * CHAN_PER_POS
SIGMA = 0.5
NCORES = 8
BL = B // NCORES
POS_PER_GROUP = 4
F32 = mybir.dt.float32

_cache = {}


def _geometry(offset):
    off_round = np.round(offset)  # round-half-even, matches jnp.round
    oxy = off_round.astype(np.int64)
    frac = (offset - off_round).astype(np.float32)

    coords = (np.arange(3, dtype=np.float32) - np.float32(1.0))
    dx = coords[None, :] + frac[:, 0:1]
    dy = coords[None, :] + frac[:, 1:2]
    inv = np.float32(1.0 / (2.0 * SIGMA * SIGMA))
    gx = np.exp(-(dx * dx) * inv).astype(np.float32)
    gy = np.exp(-(dy * dy) * inv).astype(np.float32)
    wx = gx / gx.sum(axis=1, keepdims=True)
    wy = gy / gy.sum(axis=1, keepdims=True)

    pos = {}
    for p in range(NUM_POS):
        ox, oy = int(oxy[p, 0]), int(oxy[p, 1])
        vy0, vy1 = max(0, oy), min(H, H + oy)
        vx0, vx1 = max(0, ox), min(W, W + ox)
        if vy1 <= vy0 or vx1 <= vx0:
            continue
        pos[p] = dict(
            p=p, ox=ox, oy=oy,
            vy0=vy0, vy1=vy1,
            sy0=vy0 - oy, sx0=vx0 - ox,
            wv=vx1 - vx0,
            by0=max(0, vy0 - 1), by1=min(H, vy1 + 1),
            bx0=max(0, vx0 - 1), bx1=min(W, vx1 + 1),
        )

    # strict same-ox groups, sorted by oy inside so row unions stay tight
    by_ox = {}
    for p, m in sorted(pos.items(), key=lambda kv: (kv[1]["ox"], kv[1]["oy"])):
        by_ox.setdefault(m["ox"], []).append(m)

    groups = []
    for ox in sorted(by_ox):
        mem = by_ox[ox]
        for i in range(0, len(mem), POS_PER_GROUP):
            members = mem[i:i + POS_PER_GROUP]
            gby0 = min(m["by0"] for m in members)
            gby1 = max(m["by1"] for m in members)
            sx0 = members[0]["sx0"]
            wv = members[0]["wv"]
            ud0 = max(0, sx0 - 2)
            ud1 = min(W, sx0 + wv + 2)
            groups.append(dict(
                members=members, ox=ox, gby0=gby0, gby1=gby1,
                sx0=sx0, wv=wv, ud0=ud0, ud1=ud1,
                bx0=members[0]["bx0"], bx1=members[0]["bx1"],
            ))

    ng = len(groups)
    taps = np.zeros((128, max(ng, 1) * 6), dtype=np.float32)
    for g, grp in enumerate(groups):
        for i, m in enumerate(grp["members"]):
            rows = slice(i * 32, (i + 1) * 32)
            for k in range(3):
                taps[rows, g * 6 + k] = wy[m["p"], k]
                taps[rows, g * 6 + 3 + k] = wx[m["p"], k]
    return groups, taps


def _build(groups, n_tap_cols):
    nc = bacc.Bacc("TRN2", target_bir_lowering=False, debug=False,
                   num_devices=NCORES)
    inp_d = nc.dram_tensor("inp", [BL, C, H, W], F32, kind="ExternalInput")
    taps_d = nc.dram_tensor("taps", [128, n_tap_cols], F32, kind="ExternalInput")
    out_d = nc.dram_tensor("out", [BL, C, H, W], F32, kind="ExternalOutput")

    mult = mybir.AluOpType.mult
    add = mybir.AluOpType.add

    with tile.TileContext(nc) as tc:
        with ExitStack() as ctx:
            dpool = ctx.enter_context(tc.tile_pool(name="dpool", bufs=3))
            tpool = ctx.enter_context(tc.tile_pool(name="tpool", bufs=2))
            opool = ctx.enter_context(tc.tile_pool(name="opool", bufs=2))
            cpool = ctx.enter_context(tc.tile_pool(name="cpool", bufs=1))

            taps_t = cpool.tile([128, n_tap_cols], F32, tag="taps")
            nc.sync.dma_start(taps_t[:], taps_d.ap()[:, :])

            def tap(g, k):
                return taps_t[:, g * 6 + k:g * 6 + k + 1]

            for g, grp in enumerate(groups):
                gby0, gby1 = grp["gby0"], grp["gby1"]
                bg = gby1 - gby0
                drows = bg + 2
                ox = grp["ox"]
                sx0, wv, ud0, ud1 = grp["sx0"], grp["wv"], grp["ud0"], grp["ud1"]
                wd = ud1 - ud0
                wt = wd + 4
                bx0, bx1 = grp["bx0"], grp["bx1"]
                wb = bx1 - bx0

                d_t = dpool.tile([128, drows * W], F32, tag="D")
                d3 = d_t[:].rearrange("q (r c) -> q r c", c=W)
                # zero the compute window (rows outside each member's valid
                # range must read as zero; DMA overwrites the valid rows)
                nc.gpsimd.memset(d3[:, :, ud0:ud1], 0.0)

                for i, m in enumerate(grp["members"]):
                    hv = m["vy1"] - m["vy0"]
                    r0 = 1 + m["vy0"] - gby0
                    q0 = i * 32
                    dst = d_t[q0:q0 + 32, r0 * W:(r0 + hv) * W]
                    src = inp_d.ap()[:, 8 * m["p"]:8 * m["p"] + 8,
                                     m["sy0"]:m["sy0"] + hv, :]
                    src = src.rearrange("b ch r c -> b ch (r c)")
                    eng = nc.sync if i % 2 == 0 else nc.scalar
                    eng.dma_start(dst, src)

                # the DMA wrote full-width rows; re-zero the masked columns
                # that fall inside the compute window (<= 2 cols each side)
                if sx0 > ud0:
                    nc.gpsimd.memset(d3[:, :, ud0:sx0], 0.0)
                if ud1 > sx0 + wv:
                    nc.gpsimd.memset(d3[:, :, sx0 + wv:ud1], 0.0)

                # y-conv: T[tr, 2+j] = sum_ky wy[ky] * D[tr+ky, ud0+j]
                t_t = tpool.tile([128, bg * wt], F32, tag="T")
                t3 = t_t[:].rearrange("q (r c) -> q r c", c=wt)
                nc.gpsimd.memset(t3[:, :, 0:2], 0.0)
                nc.gpsimd.memset(t3[:, :, wt - 2:wt], 0.0)
                tdat = t3[:, :, 2:2 + wd]
                nc.scalar.mul(tdat, d3[:, 0:bg, ud0:ud1], tap(g, 0))
                nc.vector.scalar_tensor_tensor(
                    tdat, d3[:, 1:1 + bg, ud0:ud1], tap(g, 1), tdat, mult, add)
                nc.vector.scalar_tensor_tensor(
                    tdat, d3[:, 2:2 + bg, ud0:ud1], tap(g, 2), tdat, mult, add)

                # x-conv with the x-shift folded into the read offsets:
                # O[tr, x] = sum_kx wx[kx] * T[tr, (x - ox + kx - 1) - ud0 + 2]
                o_t = opool.tile([128, bg * W], F32, tag="O")
                o3 = o_t[:].rearrange("q (r c) -> q r c", c=W)
                if bx0 > 0:
                    nc.gpsimd.memset(o3[:, :, 0:bx0], 0.0)
                if bx1 < W:
                    nc.gpsimd.memset(o3[:, :, bx1:W], 0.0)
                odat = o3[:, :, bx0:bx1]
                c0 = bx0 - ox - 1 - ud0 + 2
                nc.scalar.mul(odat, t3[:, :, c0:c0 + wb], tap(g, 3))
                nc.vector.scalar_tensor_tensor(
                    odat, t3[:, :, c0 + 1:c0 + 1 + wb], tap(g, 4), odat,
                    mult, add)
                nc.vector.scalar_tensor_tensor(
                    odat, t3[:, :, c0 + 2:c0 + 2 + wb], tap(g, 5), odat,
                    mult, add)

                # full-width row-band writes (contiguous in HBM; off-band
                # cols are zero)
                for i, m in enumerate(grp["members"]):
                    r0, r1 = m["by0"] - gby0, m["by1"] - gby0
                    q0 = i * 32
                    src = o_t[q0:q0 + 32, r0 * W:r1 * W]
                    dst = out_d.ap()[:, 8 * m["p"]:8 * m["p"] + 8,
                                     m["by0"]:m["by1"], :]
                    dst = dst.rearrange("b ch r c -> b ch (r c)")
                    eng = nc.sync if i % 2 == 1 else nc.scalar
                    eng.dma_start(dst, src)

    nc.compile()
    return nc


def kernel(inp, offset):
    inp = np.ascontiguousarray(inp, dtype=np.float32)
    offset = np.ascontiguousarray(offset, dtype=np.float32)
    assert inp.shape == (B, C, H, W), inp.shape

    key = offset.tobytes()
    if key not in _cache:
        groups, taps = _geometry(offset)
        nc = _build(groups, taps.shape[1])
        _cache[key] = (nc, taps)
    nc, taps = _cache[key]

    in_maps = [{"inp": inp[c * BL:(c + 1) * BL], "taps": taps}
               for c in range(NCORES)]
    trace = os.environ.get("KERNEL_TRACE", "") == "1"
    res = run_bass_kernel_spmd(nc, in_maps, core_ids=list(range(NCORES)),
                               trace=trace)
    if trace:
        print(f"HW exec time: {res.exec_time_ns} ns "
              f"(mean {res.mean_exec_time_ns})")
        kernel.last_exec_time_ns = res.exec_time_ns
    out = np.concatenate([res.results[c]["out"] for c in range(NCORES)],
                         axis=0)
    return out
